# revision 26
# baseline (speedup 1.0000x reference)
"""Trainium2 Bass kernel for nn_EquivariantGating.

Reference computation (after dead-code elimination of out1/out2):
    s : (z=512, d=256)   v : (z, 3)          [m1 = 1]
    out0[z,w] = pw0 * ( sum_{u,v} s[z,u] s[z,v] W1[u,v,w]
                        + INV_SQRT3 * |v_z|^2 * W4[w] )
    lin = out0 @ WL / sqrt(d_h)              -> (z, 2)
    return lin[:, :1], lin[:, 1:]  reshaped to (B, N, 1)

Because the final linear has only d_out=2 columns and everything in between
is linear in the weights, the d_h=256 hidden axis folds away on the host:
    Weff[u,v,j] = scale * sum_w W1[u,v,w] WL[w,j]      (256, 256, 2)
    c[j]        = scale * INV_SQRT3 * sum_w W4[w] WL[w,j]
    lin[z,j]    = s_z^T Weff_j s_z + c[j] * |v_z|^2
The device evaluates the batched quadratic form, data-parallel over z
across 8 NeuronCores (64 nodes per core):
    PE : t_j[z,v] = sum_u sT[u,z] Weff_j[u,v]   (lhsT = sT stationary,
         rhs = Weff_j moving, fp32, PSUM accumulate over two 128-row
         u blocks; preceded by dummy matmuls that release the HAM clock
         gate while weights stream in)
    DVE: lin[z,j] = sum_v s[z,v] * t_j[z,v] + c_j*|v_z|^2 via
         scalar_tensor_tensor accumulate.
Inputs are issued on four different engines' DMA queues in parallel.
"""

from contextlib import ExitStack

import numpy as np

import concourse.bass as bass
import concourse.mybir as mybir
from concourse.bass_utils import run_bass_kernel_spmd

F32 = mybir.dt.float32
BF16 = mybir.dt.bfloat16
MULT = mybir.AluOpType.mult

N_CORES = 8
B, N = 2, 256
Z = B * N              # 512 flattened nodes
ZL = Z // N_CORES      # 64 nodes per core
D = 256                # scalar channels
INV_SQRT3 = 0.5773502691896258
N_WARMUP = 30          # dummy matmuls to release the PE HAM clock gate

_CACHE = {}


def build_nc():
    nc = bass.Bass()
    w = nc.declare_dram_parameter("w", [128, 4 * D], F32, isOutput=False)
    st = nc.declare_dram_parameter("st", [128, 2 * ZL], F32, isOutput=False)
    snv = nc.declare_dram_parameter("snv", [ZL, D + 5], F32, isOutput=False)
    out = nc.declare_dram_parameter("out", [ZL, 4], F32, isOutput=True)

    with ExitStack() as ctx:
        e = ctx.enter_context
        W = e(nc.sbuf_tensor([128, 4 * D], F32))     # Weff [u_p, (j, kb, v)]
        ST = e(nc.sbuf_tensor([128, 2 * ZL], F32))   # sT: chan kb*128+p at col kb*64+z
        SNV = e(nc.sbuf_tensor([ZL, D + 5], F32))    # [s | v | c0 | c1]
        DUMMY = e(nc.sbuf_tensor([128, 256], BF16))  # warmup operand
        SCR0 = e(nc.sbuf_tensor([ZL, D], F32))       # stt elementwise scratch
        SCR1 = e(nc.sbuf_tensor([ZL, D], F32))
        SCRV0 = e(nc.sbuf_tensor([ZL, 3], F32))
        SCRV1 = e(nc.sbuf_tensor([ZL, 3], F32))
        RT = e(nc.sbuf_tensor([ZL, 4], F32))         # [sum_v s*t_j | c_j*|v|^2]
        PT0 = e(nc.psum_tensor([ZL, D], F32))        # separate banks
        PT1 = e(nc.psum_tensor([ZL, D], F32))
        PDUM = e(nc.psum_tensor([128, 2 * D], F32))  # warmup sink
        w0sem = e(nc.semaphore("w0sem"))
        w1sem = e(nc.semaphore("w1sem"))
        stsem = e(nc.semaphore("stsem"))
        snvsem = e(nc.semaphore("snvsem"))
        dsem = e(nc.semaphore("dsem"))
        tvsem = e(nc.semaphore("tvsem"))
        rsem = e(nc.semaphore("rsem"))
        vsem = e(nc.semaphore("vsem"))
        pesem = e(nc.semaphore("pesem"))
        dma_out = e(nc.semaphore("dma_out"))

        with nc.Block() as block:
            PTs = (PT0, PT1)
            SCRs, SCRVs = (SCR0, SCR1), (SCRV0, SCRV1)
            S, V = SNV[:, 0:D], SNV[:, D:D + 3]

            @block.sync
            def _(sync):
                # one HWDGE queue — fanning out across engine queues
                # measurably contends (both the gpsimd-SWDGE and the
                # sync+scalar-HWDGE splits benched slower).
                sync.dma_start(out=W[:, 0:2 * D],
                               in_=w[:, 0:2 * D]).then_inc(w0sem, 16)
                sync.dma_start(out=ST[:, :], in_=st[:, :]).then_inc(stsem, 16)
                sync.dma_start(out=W[:, 2 * D:4 * D],
                               in_=w[:, 2 * D:4 * D]).then_inc(w1sem, 16)
                sync.dma_start(out=SNV[:, :], in_=snv[:, :]).then_inc(snvsem, 16)
                sync.wait_ge(rsem, 2)
                sync.wait_ge(tvsem, 2)
                # completion is guaranteed by the Block-exit dge drain; no
                # explicit wait on dma_out needed.
                sync.dma_start(out=out[:, :], in_=RT[:, :]).then_inc(dma_out, 16)

            @block.gpsimd
            def _(gpsimd):
                gpsimd.memset(DUMMY[:, :], 1.0).then_inc(dsem, 1)

            @block.tensor
            def _(tensor):
                # dummy matmuls keep the PE busy from (nearly) its first
                # cycle so the HAM clock gate opens (~4us of sustained
                # activity) before the real matmuls.
                tensor.wait_ge(dsem, 1)
                for _ in range(N_WARMUP):
                    tensor.matmul(PDUM[:, 0:128], DUMMY[:, 0:128],
                                  DUMMY[:, 128:256], start=True, stop=True)
                tensor.wait_ge(stsem, 16)
                for j in range(2):
                    tensor.wait_ge((w0sem, w1sem)[j], 16)
                    for kb in range(2):
                        mm = tensor.matmul(
                            PTs[j][:, :], ST[:, kb * ZL:(kb + 1) * ZL],
                            W[:, (2 * j + kb) * D:(2 * j + kb + 1) * D],
                            start=(kb == 0), stop=(kb == 1))
                    mm.then_inc(pesem, 1)

            @block.vector
            def _(vector):
                vector.wait_ge(snvsem, 16)
                vector.wait_ge(pesem, 1)
                vector.scalar_tensor_tensor(
                    out=SCRs[0][:, :], in0=S, scalar=1.0,
                    in1=PTs[0][:, :], op0=MULT, op1=MULT,
                    accum_out=RT[:, 0:1]).then_inc(rsem, 1)
                vector.wait_ge(snvsem, 16)
                for j in range(2):
                    # RT[:, 2+j] = sum_i (v_i * c_j) * v_i = c_j * |v|^2
                    vector.scalar_tensor_tensor(
                        out=SCRVs[j][:, :], in0=V,
                        scalar=SNV[:, D + 3 + j:D + 4 + j], in1=V,
                        op0=MULT, op1=MULT,
                        accum_out=RT[:, 2 + j:3 + j]).then_inc(tvsem, 1)
                vector.wait_ge(pesem, 2)
                vector.scalar_tensor_tensor(
                    out=SCRs[1][:, :], in0=S, scalar=1.0,
                    in1=PTs[1][:, :], op0=MULT, op1=MULT,
                    accum_out=RT[:, 1:2]).then_inc(rsem, 1)

    # Drop the framework's post-const all-engine barrier from the preamble:
    # this kernel never reads the const tiles it protects, every cross-engine
    # dependency has an explicit semaphore, and removing it lets the input
    # DMAs and the PE warmup start ~1.5us earlier.
    main = nc.m.functions[0].blocks[0]
    for i in [i for i in main.instructions
              if type(i).__name__ == "InstDrain"
              or (type(i).__name__ == "InstEventSemaphore"
                  and str(getattr(i, "name", "")).startswith("barrier_"))]:
        main.instructions.remove(i)
    return nc


def _prepare(vectors, scalars, W1, W4, WL):
    d = scalars.shape[-1]
    d_h = W1.shape[-1]
    m1 = vectors.shape[-1] // 3
    pw0 = (1.0 / (d * d + m1 * m1)) ** 0.5
    scale = pw0 / np.sqrt(d_h)
    WL64 = WL.astype(np.float64)
    Weff = scale * (W1.astype(np.float64).reshape(d * d, d_h) @ WL64)
    # [u, v, j] -> [p, (j, kb, v)] with u = kb*128 + p
    wparam = np.ascontiguousarray(
        Weff.reshape(d, d, 2).transpose(2, 0, 1)      # j, u, v
        .reshape(2, 2, 128, d)                        # j, kb, p, v
        .transpose(2, 0, 1, 3).reshape(128, 4 * d)    # p, (j kb v)
    ).astype(np.float32)
    c = (scale * INV_SQRT3) * (W4.astype(np.float64).reshape(d_h) @ WL64)
    s = scalars.reshape(Z, d).astype(np.float32)
    v = vectors.reshape(Z, 3 * m1).astype(np.float32)
    in_maps = []
    for i in range(N_CORES):
        sl = slice(i * ZL, (i + 1) * ZL)
        s_loc, v_loc = s[sl], v[sl]
        st = np.ascontiguousarray(
            s_loc.T.reshape(2, 128, ZL).transpose(1, 0, 2).reshape(128, 2 * ZL))
        ones = np.ones((ZL, 1), np.float64)
        snv = np.concatenate(
            [s_loc, v_loc, c[0] * ones, c[1] * ones], axis=1
        ).astype(np.float32)
        in_maps.append({"w": wparam, "st": st,
                        "snv": np.ascontiguousarray(snv)})
    return in_maps


def kernel(vectors, scalars, W1, W2a, W2b, W3a, W3b, W4, WL):
    in_maps = _prepare(vectors, scalars, W1, W4, WL)
    if "nc" not in _CACHE:
        _CACHE["nc"] = build_nc()
    res = run_bass_kernel_spmd(_CACHE["nc"], in_maps, list(range(N_CORES)))
    rt = np.concatenate([res.results[i]["out"] for i in range(N_CORES)],
                        axis=0)                      # (Z, 4)
    lin = (rt[:, 0:2] + rt[:, 2:4]).astype(np.float32)  # (Z, 2)
    m_eqv = np.ascontiguousarray(lin[:, :1].reshape(B, N, 1))
    m_inv = np.ascontiguousarray(lin[:, 1:].reshape(B, N, 1))
    return (m_eqv, m_inv)


# revision 27
# speedup vs baseline: 1.0304x; 1.0304x over previous
"""Trainium2 Bass kernel for nn_EquivariantGating.

Reference computation (after dead-code elimination of out1/out2):
    s : (z=512, d=256)   v : (z, 3)          [m1 = 1]
    out0[z,w] = pw0 * ( sum_{u,v} s[z,u] s[z,v] W1[u,v,w]
                        + INV_SQRT3 * |v_z|^2 * W4[w] )
    lin = out0 @ WL / sqrt(d_h)              -> (z, 2)
    return lin[:, :1], lin[:, 1:]  reshaped to (B, N, 1)

Because the final linear has only d_out=2 columns and everything in between
is linear in the weights, the d_h=256 hidden axis folds away on the host:
    Weff[u,v,j] = scale * sum_w W1[u,v,w] WL[w,j]      (256, 256, 2)
    c[j]        = scale * INV_SQRT3 * sum_w W4[w] WL[w,j]
    lin[z,j]    = s_z^T Weff_j s_z + c[j] * |v_z|^2
The device evaluates the batched quadratic form, data-parallel over z
across 8 NeuronCores (64 nodes per core):
    PE : t_j[z,v] = sum_u sT[u,z] Weff_j[u,v]   (lhsT = sT stationary,
         rhs = Weff_j moving, fp32, PSUM accumulate over two 128-row
         u blocks; preceded by dummy matmuls that release the HAM clock
         gate while weights stream in)
    DVE: lin[z,j] = sum_v s[z,v] * t_j[z,v] + c_j*|v_z|^2 via
         scalar_tensor_tensor accumulate.
Inputs are issued on four different engines' DMA queues in parallel.
"""

from contextlib import ExitStack

import numpy as np

import concourse.bass as bass
import concourse.mybir as mybir
from concourse.bass_utils import run_bass_kernel_spmd

F32 = mybir.dt.float32
BF16 = mybir.dt.bfloat16
MULT = mybir.AluOpType.mult

N_CORES = 8
B, N = 2, 256
Z = B * N              # 512 flattened nodes
ZL = Z // N_CORES      # 64 nodes per core
D = 256                # scalar channels
INV_SQRT3 = 0.5773502691896258
N_WARMUP = 36          # dummy matmuls to release the PE HAM clock gate

_CACHE = {}


def build_nc():
    nc = bass.Bass()
    w = nc.declare_dram_parameter("w", [128, 4 * D], F32, isOutput=False)
    st = nc.declare_dram_parameter("st", [128, 2 * ZL], F32, isOutput=False)
    snv = nc.declare_dram_parameter("snv", [ZL, D + 5], F32, isOutput=False)
    out = nc.declare_dram_parameter("out", [ZL, 4], F32, isOutput=True)

    with ExitStack() as ctx:
        e = ctx.enter_context
        W = e(nc.sbuf_tensor([128, 4 * D], F32))     # Weff [u_p, (j, kb, v)]
        ST = e(nc.sbuf_tensor([128, 2 * ZL], F32))   # sT: chan kb*128+p at col kb*64+z
        SNV = e(nc.sbuf_tensor([ZL, D + 5], F32))    # [s | v | c0 | c1]
        DUMMY = e(nc.sbuf_tensor([128, 256], BF16))  # warmup operand
        SCR0 = e(nc.sbuf_tensor([ZL, D], F32))       # stt elementwise scratch
        SCR1 = e(nc.sbuf_tensor([ZL, D], F32))
        SCRV0 = e(nc.sbuf_tensor([ZL, 3], F32))
        SCRV1 = e(nc.sbuf_tensor([ZL, 3], F32))
        RT = e(nc.sbuf_tensor([ZL, 4], F32))         # [sum_v s*t_j | c_j*|v|^2]
        PT0 = e(nc.psum_tensor([ZL, D], F32))        # separate banks
        PT1 = e(nc.psum_tensor([ZL, D], F32))
        PDUM = e(nc.psum_tensor([128, 2 * D], F32))  # warmup sink
        w0sem = e(nc.semaphore("w0sem"))
        w1sem = e(nc.semaphore("w1sem"))
        stsem = e(nc.semaphore("stsem"))
        snvsem = e(nc.semaphore("snvsem"))
        dsem = e(nc.semaphore("dsem"))
        tvsem = e(nc.semaphore("tvsem"))
        rsem = e(nc.semaphore("rsem"))
        vsem = e(nc.semaphore("vsem"))
        pesem = e(nc.semaphore("pesem"))
        dma_out = e(nc.semaphore("dma_out"))

        with nc.Block() as block:
            PTs = (PT0, PT1)
            SCRs, SCRVs = (SCR0, SCR1), (SCRV0, SCRV1)
            S, V = SNV[:, 0:D], SNV[:, D:D + 3]

            @block.sync
            def _(sync):
                # one HWDGE queue — fanning out across engine queues
                # measurably contends (both the gpsimd-SWDGE and the
                # sync+scalar-HWDGE splits benched slower).
                sync.dma_start(out=W[:, 0:2 * D],
                               in_=w[:, 0:2 * D]).then_inc(w0sem, 16)
                sync.dma_start(out=ST[:, :], in_=st[:, :]).then_inc(stsem, 16)
                sync.dma_start(out=W[:, 2 * D:4 * D],
                               in_=w[:, 2 * D:4 * D]).then_inc(w1sem, 16)
                sync.dma_start(out=SNV[:, :], in_=snv[:, :]).then_inc(snvsem, 16)
                sync.wait_ge(rsem, 2)
                sync.wait_ge(tvsem, 2)
                # completion is guaranteed by the Block-exit dge drain; no
                # explicit wait on dma_out needed.
                sync.dma_start(out=out[:, :], in_=RT[:, :]).then_inc(dma_out, 16)

            @block.gpsimd
            def _(gpsimd):
                gpsimd.memset(DUMMY[:, :], 1.0).then_inc(dsem, 1)

            @block.tensor
            def _(tensor):
                # dummy matmuls keep the PE busy from (nearly) its first
                # cycle so the HAM clock gate opens (~4us of sustained
                # activity) before the real matmuls.
                tensor.wait_ge(dsem, 1)
                for _ in range(N_WARMUP):
                    tensor.matmul(PDUM[:, 0:128], DUMMY[:, 0:128],
                                  DUMMY[:, 128:256], start=True, stop=True)
                tensor.wait_ge(stsem, 16)
                for j in range(2):
                    tensor.wait_ge((w0sem, w1sem)[j], 16)
                    for kb in range(2):
                        mm = tensor.matmul(
                            PTs[j][:, :], ST[:, kb * ZL:(kb + 1) * ZL],
                            W[:, (2 * j + kb) * D:(2 * j + kb + 1) * D],
                            start=(kb == 0), stop=(kb == 1))
                    mm.then_inc(pesem, 1)

            @block.vector
            def _(vector):
                vector.wait_ge(snvsem, 16)
                vector.wait_ge(pesem, 1)
                vector.scalar_tensor_tensor(
                    out=SCRs[0][:, :], in0=S, scalar=1.0,
                    in1=PTs[0][:, :], op0=MULT, op1=MULT,
                    accum_out=RT[:, 0:1]).then_inc(rsem, 1)
                vector.wait_ge(snvsem, 16)
                for j in range(2):
                    # RT[:, 2+j] = sum_i (v_i * c_j) * v_i = c_j * |v|^2
                    vector.scalar_tensor_tensor(
                        out=SCRVs[j][:, :], in0=V,
                        scalar=SNV[:, D + 3 + j:D + 4 + j], in1=V,
                        op0=MULT, op1=MULT,
                        accum_out=RT[:, 2 + j:3 + j]).then_inc(tvsem, 1)
                vector.wait_ge(pesem, 2)
                vector.scalar_tensor_tensor(
                    out=SCRs[1][:, :], in0=S, scalar=1.0,
                    in1=PTs[1][:, :], op0=MULT, op1=MULT,
                    accum_out=RT[:, 1:2]).then_inc(rsem, 1)

    # Drop the framework's post-const all-engine barrier from the preamble:
    # this kernel never reads the const tiles it protects, every cross-engine
    # dependency has an explicit semaphore, and removing it lets the input
    # DMAs and the PE warmup start ~1.5us earlier.
    main = nc.m.functions[0].blocks[0]
    for i in [i for i in main.instructions
              if type(i).__name__ == "InstDrain"
              or (type(i).__name__ == "InstEventSemaphore"
                  and str(getattr(i, "name", "")).startswith("barrier_"))]:
        main.instructions.remove(i)
    # Move the DUMMY warmup memset (the only Pool memset with a semaphore
    # rider) ahead of the framework const memsets so the PE warmup can start
    # as early as possible.
    pool_ms = [i for i in nc.m.functions[0].blocks
               if i.name.endswith("_Pool_67") or "_Pool_" in i.name]
    dummy_ms = None
    for b in nc.m.functions[0].blocks[1:]:
        for i in list(b.instructions):
            if type(i).__name__ == "InstMemset" and i.sync_info is not None \
                    and i.sync_info.on_update:
                dummy_ms = i
                b.instructions.remove(i)
                break
        if dummy_ms is not None:
            break
    assert dummy_ms is not None
    first_const = next(idx for idx, i in enumerate(main.instructions)
                       if type(i).__name__ == "InstMemset")
    main.instructions.insert(first_const, dummy_ms)
    return nc


def _prepare(vectors, scalars, W1, W4, WL):
    d = scalars.shape[-1]
    d_h = W1.shape[-1]
    m1 = vectors.shape[-1] // 3
    pw0 = (1.0 / (d * d + m1 * m1)) ** 0.5
    scale = pw0 / np.sqrt(d_h)
    WL64 = WL.astype(np.float64)
    Weff = scale * (W1.astype(np.float64).reshape(d * d, d_h) @ WL64)
    # [u, v, j] -> [p, (j, kb, v)] with u = kb*128 + p
    wparam = np.ascontiguousarray(
        Weff.reshape(d, d, 2).transpose(2, 0, 1)      # j, u, v
        .reshape(2, 2, 128, d)                        # j, kb, p, v
        .transpose(2, 0, 1, 3).reshape(128, 4 * d)    # p, (j kb v)
    ).astype(np.float32)
    c = (scale * INV_SQRT3) * (W4.astype(np.float64).reshape(d_h) @ WL64)
    s = scalars.reshape(Z, d).astype(np.float32)
    v = vectors.reshape(Z, 3 * m1).astype(np.float32)
    in_maps = []
    for i in range(N_CORES):
        sl = slice(i * ZL, (i + 1) * ZL)
        s_loc, v_loc = s[sl], v[sl]
        st = np.ascontiguousarray(
            s_loc.T.reshape(2, 128, ZL).transpose(1, 0, 2).reshape(128, 2 * ZL))
        ones = np.ones((ZL, 1), np.float64)
        snv = np.concatenate(
            [s_loc, v_loc, c[0] * ones, c[1] * ones], axis=1
        ).astype(np.float32)
        in_maps.append({"w": wparam, "st": st,
                        "snv": np.ascontiguousarray(snv)})
    return in_maps


def kernel(vectors, scalars, W1, W2a, W2b, W3a, W3b, W4, WL):
    in_maps = _prepare(vectors, scalars, W1, W4, WL)
    if "nc" not in _CACHE:
        _CACHE["nc"] = build_nc()
    res = run_bass_kernel_spmd(_CACHE["nc"], in_maps, list(range(N_CORES)))
    rt = np.concatenate([res.results[i]["out"] for i in range(N_CORES)],
                        axis=0)                      # (Z, 4)
    lin = (rt[:, 0:2] + rt[:, 2:4]).astype(np.float32)  # (Z, 2)
    m_eqv = np.ascontiguousarray(lin[:, :1].reshape(B, N, 1))
    m_inv = np.ascontiguousarray(lin[:, 1:].reshape(B, N, 1))
    return (m_eqv, m_inv)
